# revision 16
# baseline (speedup 1.0000x reference)
"""Mixtral-style MoE block on 8 Trainium2 NeuronCores (expert-parallel).

Problem (hardcoded shapes): x [2,2048,1024] f32, router_w [8,1024],
w1/w2 [8,3584,1024], w3 [8,1024,3584].  E=8 experts, top-2 routing.

Strategy (expert-parallel, per the sharding hint):
  - Host: router (softmax + top-2 + gate renorm) on the replicated router
    weights; dispatch ("all-to-all") = gather each expert's routed tokens
    into a padded [NCAP, C] buffer on the host.
  - Device (core e == expert e): the FFN GEMMs, which are >99.9% of the
    FLOPs:   h = silu(x@w2e^T) * (x@w1e^T);  y = h @ w3e^T
    computed entirely with the token dim on the matmul free axis so no
    on-device transposes are needed.  fp32r matmuls (fp22 mantissa trunc,
    1.5 PE cycles/row vs 2.0 for full fp32).
  - Host: combine (scatter-add with gate weights) + aux loss.

Self-contained: only needs numpy + the installed concourse/bass stack.
"""

import numpy as np

import concourse.bass as bass  # noqa: F401  (bass types via bacc/tile)
import concourse.tile as tile
from concourse import bacc, mybir
from concourse.bass_utils import run_bass_kernel_spmd

# ---------------------------------------------------------------- constants
E, K = 8, 2
B, T, C, F = 2, 2048, 1024, 3584
N = B * T                     # 4096 tokens
P = 128
CS = C // P                   # 8  c-subtiles
FS = F // P                   # 28 f-subtiles
NCAP = 1088                   # padded per-expert token capacity (actual max 1071)
# All tiles >= 256 wide: f32r matmul runs 1.0 PE-cycles/row at moving dim
# >= 256 but 4.0 below it (instruction_cost_v2.rs), so avoid narrow tails.
TOK_TILES = [(0, 512), (512, 288), (800, 288)]
MM_DT = mybir.dt.float32r     # PE input dtype (fp22-truncated fp32 read)
ACT_SILU = True               # use HW Silu table (not implemented in CoreSim)


def configure(dtype_name=None, ncap=None):
    """Override matmul dtype ('float32r'|'bfloat16'|'float32') / capacity."""
    global MM_DT, NCAP, TOK_TILES
    if dtype_name is not None:
        MM_DT = getattr(mybir.dt, dtype_name)
    if ncap is not None:
        NCAP = ncap
        tiles, off = [], 0
        while off < ncap:
            w = min(512, ncap - off)
            if 0 < ncap - off - w < 256:       # avoid a <256-wide tail tile
                w = (ncap - off) // 2
            tiles.append((off, w))
            off += w
        TOK_TILES = tiles

# Results of the last device run (for test harness introspection).
LAST_RESULTS = None


# ---------------------------------------------------------------- device code
def _emit_kernel(tc, xT, w1h, w2h, w3h, yT):
    """Per-core FFN: yT[:, :] = (silu(x@w2^T) * (x@w1^T)) @ w3^T, transposed.

    xT  : [C, NCAP]          tokens for this core's expert, transposed
    w1h : [FS, P, CS, P]     w1h[f,p,c,i] = w1e[f*128+i, c*128+p]   (lhsT tiles)
    w2h : [FS, P, CS, P]     same layout for w2e
    w3h : [CS, P, FS, P]     w3h[c,p,f,j] = w3e[c*128+j, f*128+p]   (lhsT tiles)
    yT  : [C, NCAP]          output, transposed
    """
    nc = tc.nc
    f32 = mybir.dt.float32

    with (
        tc.tile_pool(name="hpool", bufs=1) as hpool,
        tc.tile_pool(name="ypool", bufs=3) as ypool,
        tc.tile_pool(name="w3pool", bufs=2) as w3pool,
        tc.tile_pool(name="psum", bufs=8, space="PSUM") as pspool,
    ):
        # h^T resident for all tokens: [F(part-tiled), NCAP]
        h_sb = hpool.tile([P, FS, NCAP], MM_DT, name="h_sb")

        # Prefetch the first w3 column-block so phase 2 starts immediately.
        w3_first = w3pool.tile([P, FS, P], MM_DT, tag="w3", name="w3_sb")
        nc.sync.dma_start(w3_first[:], w3h[0])

        # ---------------- phase 1: h = silu(x@w2^T) * (x@w1^T) ----------------
        with (
            tc.tile_pool(name="xpool", bufs=1) as xpool,
            tc.tile_pool(name="wpool", bufs=2) as wpool,
        ):
            xT_sb = xpool.tile([P, CS, NCAP], MM_DT, name="xT_sb")
            xT_r = xT.rearrange("(cs p) n -> p cs n", p=P)
            # One DMA per c-subtile so the first matmuls only wait for 1/8
            # of x (Tile tracks slice-level deps).
            for c in range(CS):
                nc.sync.dma_start(xT_sb[:, c], xT_r[:, c])

            for f in range(FS):
                w2_sb = wpool.tile([P, CS, P], MM_DT, tag="w2", name="w2_sb")
                nc.sync.dma_start(w2_sb[:], w2h[f])
                w1_sb = wpool.tile([P, CS, P], MM_DT, tag="w1", name="w1_sb")
                nc.sync.dma_start(w1_sb[:], w1h[f])

                # One PSUM bank per token tile per gemm; each 128x128 weight
                # tile stays stationary while all NCAP token columns stream.
                ps_a = [
                    pspool.tile([P, 512], f32, tag="ps", name="ps_a")[:, :tw]
                    for _, tw in TOK_TILES
                ]
                ps_b = [
                    pspool.tile([P, 512], f32, tag="ps", name="ps_b")[:, :tw]
                    for _, tw in TOK_TILES
                ]
                for w_sb, ps in ((w2_sb, ps_a), (w1_sb, ps_b)):
                    for c in range(CS):
                        for ti, (t0, tw) in enumerate(TOK_TILES):
                            nc.tensor.matmul(
                                ps[ti], w_sb[:, c], xT_sb[:, c, t0 : t0 + tw],
                                start=(c == 0), stop=(c == CS - 1),
                            )
                for ti, (t0, tw) in enumerate(TOK_TILES):
                    hs = h_sb[:, f, t0 : t0 + tw]
                    if ACT_SILU:
                        nc.scalar.activation(
                            hs, ps_a[ti], mybir.ActivationFunctionType.Silu
                        )
                        nc.vector.tensor_mul(out=hs, in0=hs, in1=ps_b[ti])
                    else:  # CoreSim-compatible: silu(a) = a * sigmoid(a)
                        nc.scalar.activation(
                            hs, ps_a[ti], mybir.ActivationFunctionType.Sigmoid
                        )
                        nc.vector.tensor_mul(out=hs, in0=hs, in1=ps_a[ti])
                        nc.vector.tensor_mul(out=hs, in0=hs, in1=ps_b[ti])

        # ---------------- phase 2: y = h @ w3^T ----------------
        if True:
            for c in range(CS):
                if c == 0:
                    w3_sb = w3_first
                else:
                    w3_sb = w3pool.tile([P, FS, P], MM_DT, tag="w3", name="w3_sb")
                    nc.sync.dma_start(w3_sb[:], w3h[c])
                ps_y = [
                    pspool.tile([P, 512], f32, tag="ps", name="ps_y")[:, :tw]
                    for _, tw in TOK_TILES
                ]
                for f in range(FS):
                    for ti, (t0, tw) in enumerate(TOK_TILES):
                        nc.tensor.matmul(
                            ps_y[ti], w3_sb[:, f], h_sb[:, f, t0 : t0 + tw],
                            start=(f == 0), stop=(f == FS - 1),
                        )
                for ti, (t0, tw) in enumerate(TOK_TILES):
                    y_st = ypool.tile([P, 512], f32, tag="y", name="y_st")[:, :tw]
                    nc.vector.tensor_copy(y_st, ps_y[ti])
                    nc.sync.dma_start(yT[c * P : (c + 1) * P, t0 : t0 + tw], y_st)


def build_module(loop=1):
    """Build + compile the per-core Bass module.  loop>1 wraps the body in an
    on-device For_i repeat (identical work each iteration) for benchmarking."""
    nc = bacc.Bacc("TRN2", target_bir_lowering=False, debug=False)
    xT = nc.dram_tensor("xT", [C, NCAP], MM_DT, kind="ExternalInput").ap()
    w1h = nc.dram_tensor("w1h", [FS, P, CS, P], MM_DT, kind="ExternalInput").ap()
    w2h = nc.dram_tensor("w2h", [FS, P, CS, P], MM_DT, kind="ExternalInput").ap()
    w3h = nc.dram_tensor("w3h", [CS, P, FS, P], MM_DT, kind="ExternalInput").ap()
    yT = nc.dram_tensor("yT", [C, NCAP], mybir.dt.float32, kind="ExternalOutput").ap()
    with tile.TileContext(nc) as tc:
        if loop > 1:
            with tc.For_i(0, loop, 1):
                _emit_kernel(tc, xT, w1h, w2h, w3h, yT)
        else:
            _emit_kernel(tc, xT, w1h, w2h, w3h, yT)
    nc.compile()
    return nc


_NC = None


def _get_module():
    global _NC
    if _NC is None:
        _NC = build_module()
    return _NC


# ---------------------------------------------------------------- host code
def _route(xf, router_w):
    """fp32 router exactly mirroring the reference ops."""
    logits = xf @ router_w.T                                   # [N, E]
    m = logits.max(-1, keepdims=True)
    ex = np.exp(logits - m)
    probs = ex / ex.sum(-1, keepdims=True)
    top2 = np.argsort(-probs, axis=-1, kind="stable")[:, :K]   # ties: lower idx first
    tp = np.take_along_axis(probs, top2, axis=-1)
    gates = (tp / tp.sum(-1, keepdims=True)).astype(np.float32)
    return probs, top2, gates


def prepare_in_maps(x, router_w, w1, w2, w3):
    """Host-side routing + dispatch: per-core input dicts and combine metadata."""
    x = np.asarray(x, dtype=np.float32)
    router_w = np.asarray(router_w, dtype=np.float32)
    w1 = np.asarray(w1, dtype=np.float32)
    w2 = np.asarray(w2, dtype=np.float32)
    w3 = np.asarray(w3, dtype=np.float32)
    mm_np = mybir.dt.np(MM_DT)

    xf = x.reshape(N, C)
    probs, top2, gates = _route(xf, router_w)

    in_maps = []
    meta = []
    for e in range(E):
        sel = top2 == e                                        # [N, K]
        rows = np.nonzero(sel.any(-1))[0]
        cnt = rows.size
        assert cnt <= NCAP, f"expert {e} count {cnt} exceeds capacity {NCAP}"
        ge = (gates * sel)[rows].sum(-1).astype(np.float32)    # gate per routed token
        xg = np.zeros((NCAP, C), np.float32)
        xg[:cnt] = xf[rows]
        xT = np.ascontiguousarray(xg.T).astype(mm_np)          # [C, NCAP]
        w1h = np.ascontiguousarray(
            w1[e].reshape(FS, P, CS, P).transpose(0, 3, 2, 1)
        ).astype(mm_np)
        w2h = np.ascontiguousarray(
            w2[e].reshape(FS, P, CS, P).transpose(0, 3, 2, 1)
        ).astype(mm_np)
        w3h = np.ascontiguousarray(
            w3[e].reshape(CS, P, FS, P).transpose(0, 3, 2, 1)
        ).astype(mm_np)
        in_maps.append({"xT": xT, "w1h": w1h, "w2h": w2h, "w3h": w3h})
        meta.append((rows, ge))
    return in_maps, meta, probs, top2


def combine(per_core_results, meta, probs, top2):
    """Host-side gather: gated scatter-add + aux loss."""
    y_flat = np.zeros((N, C), np.float32)
    for e in range(E):
        rows, ge = meta[e]
        yTe = per_core_results[e]["yT"]                        # [C, NCAP]
        y_flat[rows] += yTe[:, : rows.size].T * ge[:, None]
    y = y_flat.reshape(B, T, C)

    counts = np.bincount(top2.ravel(), minlength=E).astype(np.float32)
    fvec = counts / np.float32(N * K)
    pvec = probs.mean(0).astype(np.float32)
    aux = np.float32(E * np.sum(fvec * pvec))
    return y, aux


def kernel(x, router_w, w1, w2, w3):
    global LAST_RESULTS
    in_maps, meta, probs, top2 = prepare_in_maps(x, router_w, w1, w2, w3)
    nc = _get_module()
    res = run_bass_kernel_spmd(nc, in_maps, core_ids=list(range(E)))
    LAST_RESULTS = res
    return combine(res.results, meta, probs, top2)


# revision 21
# speedup vs baseline: 1.0268x; 1.0268x over previous
"""Mixtral-style MoE block on 8 Trainium2 NeuronCores (expert-parallel).

Problem (hardcoded shapes): x [2,2048,1024] f32, router_w [8,1024],
w1/w2 [8,3584,1024], w3 [8,1024,3584].  E=8 experts, top-2 routing.

Strategy (expert-parallel, per the sharding hint):
  - Host: router (softmax + top-2 + gate renorm) on the replicated router
    weights; dispatch ("all-to-all") = gather each expert's routed tokens
    into a padded [NCAP, C] buffer on the host.
  - Device (core e == expert e): the FFN GEMMs, which are >99.9% of the
    FLOPs:   h = silu(x@w2e^T) * (x@w1e^T);  y = h @ w3e^T
    computed entirely with the token dim on the matmul free axis so no
    on-device transposes are needed.  fp32r matmuls (fp22-truncated fp32
    read): 1.0 PE-cycles/row at moving dim >= 256, i.e. bf16 speed at
    near-fp32 accuracy (measured 2.6e-4 output rel err).
  - Host: combine (scatter-add with gate weights) + aux loss.

Self-contained: only needs numpy + the installed concourse/bass stack.
"""

import numpy as np

import concourse.bass as bass  # noqa: F401  (bass types via bacc/tile)
import concourse.tile as tile
from concourse import bacc, mybir
from concourse.bass_utils import run_bass_kernel_spmd

# ---------------------------------------------------------------- constants
E, K = 8, 2
B, T, C, F = 2, 2048, 1024, 3584
N = B * T                     # 4096 tokens
P = 128
CS = C // P                   # 8  c-subtiles
FS = F // P                   # 28 f-subtiles
NCAP = 1088                   # padded per-expert token capacity (actual max 1071)
# All tiles >= 256 wide: f32r matmul runs 1.0 PE-cycles/row at moving dim
# >= 256 but 4.0 below it (instruction_cost_v2.rs), so avoid narrow tails.
TOK_TILES = [(0, 512), (512, 288), (800, 288)]
MM_DT = mybir.dt.float32r     # PE input dtype (fp22-truncated fp32 read)
ACT_SILU = True               # use HW Silu table (not implemented in CoreSim)


def configure(dtype_name=None, ncap=None):
    """Override matmul dtype ('float32r'|'bfloat16'|'float32') / capacity."""
    global MM_DT, NCAP, TOK_TILES
    if dtype_name is not None:
        MM_DT = getattr(mybir.dt, dtype_name)
    if ncap is not None:
        NCAP = ncap
        tiles, off = [], 0
        while off < ncap:
            w = min(512, ncap - off)
            if 0 < ncap - off - w < 256 <= w:  # avoid a <256-wide tail tile
                w = (ncap - off + 1) // 2
            tiles.append((off, w))
            off += w
        TOK_TILES = tiles

# Results of the last device run (for test harness introspection).
LAST_RESULTS = None


# ---------------------------------------------------------------- device code
def _emit_kernel(tc, xT, w1h, w2h, w3h, yT):
    """Per-core FFN: yT[:, :] = (silu(x@w2^T) * (x@w1^T)) @ w3^T, transposed.

    xT  : [C, NCAP]          tokens for this core's expert, transposed
    w1h : [FS, P, CS, P]     w1h[f,p,c,i] = w1e[f*128+i, c*128+p]   (lhsT tiles)
    w2h : [FS, P, CS, P]     same layout for w2e
    w3h : [CS, P, FS, P]     w3h[c,p,f,j] = w3e[c*128+j, f*128+p]   (lhsT tiles)
    yT  : [C, NCAP]          output, transposed
    """
    nc = tc.nc
    f32 = mybir.dt.float32

    with (
        tc.tile_pool(name="hpool", bufs=1) as hpool,
        tc.tile_pool(name="ypool", bufs=3) as ypool,
        tc.tile_pool(name="w3pool", bufs=2) as w3pool,
        tc.tile_pool(name="psum", bufs=8, space="PSUM") as pspool,
    ):
        # h^T resident for all tokens: [F(part-tiled), NCAP]
        h_sb = hpool.tile([P, FS, NCAP], MM_DT, name="h_sb")

        # Prefetch the first w3 column-block so phase 2 starts immediately.
        w3_first = w3pool.tile([P, FS, P], MM_DT, tag="w3", name="w3_sb")
        nc.sync.dma_start(w3_first[:], w3h[0])

        # ---------------- phase 1: h = silu(x@w2^T) * (x@w1^T) ----------------
        with (
            tc.tile_pool(name="xpool", bufs=1) as xpool,
            tc.tile_pool(name="wpool", bufs=2) as wpool,
        ):
            xT_sb = xpool.tile([P, CS, NCAP], MM_DT, name="xT_sb")
            xT_r = xT.rearrange("(cs p) n -> p cs n", p=P)
            # One DMA per c-subtile so the first matmuls only wait for 1/8
            # of x (Tile tracks slice-level deps).
            for c in range(CS):
                nc.sync.dma_start(xT_sb[:, c], xT_r[:, c])

            for f in range(FS):
                w2_sb = wpool.tile([P, CS, P], MM_DT, tag="w2", name="w2_sb")
                nc.sync.dma_start(w2_sb[:], w2h[f])
                w1_sb = wpool.tile([P, CS, P], MM_DT, tag="w1", name="w1_sb")
                nc.sync.dma_start(w1_sb[:], w1h[f])

                # One PSUM bank per token tile per gemm; each 128x128 weight
                # tile stays stationary while all NCAP token columns stream.
                ps_a = [
                    pspool.tile([P, 512], f32, tag="ps", name="ps_a")[:, :tw]
                    for _, tw in TOK_TILES
                ]
                ps_b = [
                    pspool.tile([P, 512], f32, tag="ps", name="ps_b")[:, :tw]
                    for _, tw in TOK_TILES
                ]
                for w_sb, ps in ((w2_sb, ps_a), (w1_sb, ps_b)):
                    for c in range(CS):
                        for ti, (t0, tw) in enumerate(TOK_TILES):
                            nc.tensor.matmul(
                                ps[ti], w_sb[:, c], xT_sb[:, c, t0 : t0 + tw],
                                start=(c == 0), stop=(c == CS - 1),
                            )
                for ti, (t0, tw) in enumerate(TOK_TILES):
                    hs = h_sb[:, f, t0 : t0 + tw]
                    if ACT_SILU:
                        nc.scalar.activation(
                            hs, ps_a[ti], mybir.ActivationFunctionType.Silu
                        )
                        nc.vector.tensor_mul(out=hs, in0=hs, in1=ps_b[ti])
                    else:  # CoreSim-compatible: silu(a) = a * sigmoid(a)
                        nc.scalar.activation(
                            hs, ps_a[ti], mybir.ActivationFunctionType.Sigmoid
                        )
                        nc.vector.tensor_mul(out=hs, in0=hs, in1=ps_a[ti])
                        nc.vector.tensor_mul(out=hs, in0=hs, in1=ps_b[ti])

        # ---------------- phase 2: y = h @ w3^T ----------------
        for c in range(CS):
            if c == 0:
                w3_sb = w3_first
            else:
                w3_sb = w3pool.tile([P, FS, P], MM_DT, tag="w3", name="w3_sb")
                nc.sync.dma_start(w3_sb[:], w3h[c])
            ps_y = [
                pspool.tile([P, 512], f32, tag="ps", name="ps_y")[:, :tw]
                for _, tw in TOK_TILES
            ]
            for f in range(FS):
                for ti, (t0, tw) in enumerate(TOK_TILES):
                    nc.tensor.matmul(
                        ps_y[ti], w3_sb[:, f], h_sb[:, f, t0 : t0 + tw],
                        start=(f == 0), stop=(f == FS - 1),
                    )
            for ti, (t0, tw) in enumerate(TOK_TILES):
                y_st = ypool.tile([P, 512], f32, tag="y", name="y_st")[:, :tw]
                nc.vector.tensor_copy(y_st, ps_y[ti])
                nc.sync.dma_start(yT[c * P : (c + 1) * P, t0 : t0 + tw], y_st)


def build_module(loop=1):
    """Build + compile the per-core Bass module.  loop>1 wraps the body in an
    on-device For_i repeat (identical work each iteration) for benchmarking."""
    nc = bacc.Bacc("TRN2", target_bir_lowering=False, debug=False)
    xT = nc.dram_tensor("xT", [C, NCAP], MM_DT, kind="ExternalInput").ap()
    w1h = nc.dram_tensor("w1h", [FS, P, CS, P], MM_DT, kind="ExternalInput").ap()
    w2h = nc.dram_tensor("w2h", [FS, P, CS, P], MM_DT, kind="ExternalInput").ap()
    w3h = nc.dram_tensor("w3h", [CS, P, FS, P], MM_DT, kind="ExternalInput").ap()
    yT = nc.dram_tensor("yT", [C, NCAP], mybir.dt.float32, kind="ExternalOutput").ap()
    with tile.TileContext(nc) as tc:
        if loop > 1:
            with tc.For_i(0, loop, 1):
                _emit_kernel(tc, xT, w1h, w2h, w3h, yT)
        else:
            _emit_kernel(tc, xT, w1h, w2h, w3h, yT)
    nc.compile()
    return nc


_NC_CACHE = {}


def _get_module():
    key = (str(MM_DT), NCAP)
    if key not in _NC_CACHE:
        _NC_CACHE[key] = build_module()
    return _NC_CACHE[key]


# ---------------------------------------------------------------- host code
def _route(xf, router_w):
    """fp32 router exactly mirroring the reference ops."""
    logits = xf @ router_w.T                                   # [N, E]
    m = logits.max(-1, keepdims=True)
    ex = np.exp(logits - m)
    probs = ex / ex.sum(-1, keepdims=True)
    top2 = np.argsort(-probs, axis=-1, kind="stable")[:, :K]   # ties: lower idx first
    tp = np.take_along_axis(probs, top2, axis=-1)
    gates = (tp / tp.sum(-1, keepdims=True)).astype(np.float32)
    return probs, top2, gates


def prepare_in_maps(x, router_w, w1, w2, w3):
    """Host-side routing + dispatch: per-core input dicts and combine metadata."""
    x = np.asarray(x, dtype=np.float32)
    router_w = np.asarray(router_w, dtype=np.float32)
    w1 = np.asarray(w1, dtype=np.float32)
    w2 = np.asarray(w2, dtype=np.float32)
    w3 = np.asarray(w3, dtype=np.float32)
    mm_np = mybir.dt.np(MM_DT)

    xf = x.reshape(N, C)
    probs, top2, gates = _route(xf, router_w)

    # Capacity = max routed count, padded.  For the fixed problem inputs the
    # counts are [1071,1017,1034,1071,997,1021,1007,974] -> NCAP stays 1088;
    # recomputing keeps the kernel correct for any routing.
    max_cnt = int(np.bincount(top2.ravel(), minlength=E).max())
    if max_cnt > NCAP or NCAP - max_cnt > 128:
        configure(ncap=-(-max_cnt // 16) * 16 + 16)

    in_maps = []
    meta = []
    for e in range(E):
        sel = top2 == e                                        # [N, K]
        rows = np.nonzero(sel.any(-1))[0]
        cnt = rows.size
        assert cnt <= NCAP, f"expert {e} count {cnt} exceeds capacity {NCAP}"
        ge = (gates * sel)[rows].sum(-1).astype(np.float32)    # gate per routed token
        xg = np.zeros((NCAP, C), np.float32)
        xg[:cnt] = xf[rows]
        xT = np.ascontiguousarray(xg.T).astype(mm_np)          # [C, NCAP]
        w1h = np.ascontiguousarray(
            w1[e].reshape(FS, P, CS, P).transpose(0, 3, 2, 1)
        ).astype(mm_np)
        w2h = np.ascontiguousarray(
            w2[e].reshape(FS, P, CS, P).transpose(0, 3, 2, 1)
        ).astype(mm_np)
        w3h = np.ascontiguousarray(
            w3[e].reshape(CS, P, FS, P).transpose(0, 3, 2, 1)
        ).astype(mm_np)
        in_maps.append({"xT": xT, "w1h": w1h, "w2h": w2h, "w3h": w3h})
        meta.append((rows, ge))
    return in_maps, meta, probs, top2


def combine(per_core_results, meta, probs, top2):
    """Host-side gather: gated scatter-add + aux loss."""
    y_flat = np.zeros((N, C), np.float32)
    for e in range(E):
        rows, ge = meta[e]
        yTe = per_core_results[e]["yT"]                        # [C, NCAP]
        y_flat[rows] += yTe[:, : rows.size].T * ge[:, None]
    y = y_flat.reshape(B, T, C)

    counts = np.bincount(top2.ravel(), minlength=E).astype(np.float32)
    fvec = counts / np.float32(N * K)
    pvec = probs.mean(0).astype(np.float32)
    aux = np.float32(E * np.sum(fvec * pvec))
    return y, aux


def kernel(x, router_w, w1, w2, w3):
    global LAST_RESULTS
    in_maps, meta, probs, top2 = prepare_in_maps(x, router_w, w1, w2, w3)
    nc = _get_module()
    res = run_bass_kernel_spmd(nc, in_maps, core_ids=list(range(E)))
    LAST_RESULTS = res
    return combine(res.results, meta, probs, top2)
